# revision 28
# baseline (speedup 1.0000x reference)
"""BitNet transformer block kernel for 8 Trainium2 NeuronCores.

Sharding: data-parallel. Core c handles batch c//4, token chunk c%4 (512
query tokens). Each core computes K/V for its batch's full 2048-token
sequence (redundant KV compute instead of collectives). Host rotates the
token axis per core so every core's own tokens sit at chunk 0 -> all 8
cores run one identical SPMD program.

v2 rewrite, optimized for continuous PE occupancy (p-state ramp) and
engine balance:
 - Phase A: LN normalize done in-place on the x tile (saves SBUF), K/V/Q
   weights streamed per chunk, chunk-pipelined emission so the PE never
   waits on the LN chain.
 - Phase B: S computed into 2-bank PSUM half-tiles; row max on DVE
   (negated reduce), exp on Act straight from PSUM (no SBUF staging
   copy); V carries an extra ones column so the AV matmul emits softmax
   denominators for free; attention output stays UNNORMALIZED until one
   batched normalize at the start of phase C. One-head-deep software
   pipeline: head h's QK/softmax interleaved with head h-1's
   transpose/AV on the in-order PE queue.
 - Phase C: batched rden broadcast via matmul, streamed wo/w1/w2,
   classic LN2, fused GELU from PSUM.
"""
import sys

sys.path.insert(0, "/opt/trn_rl_repo")

import numpy as np
import ml_dtypes
from contextlib import ExitStack

import concourse.bass as bass
import concourse.bacc as bacc
import concourse.tile as tile
from concourse import mybir
from concourse.bass_utils import run_bass_kernel_spmd
from concourse.masks import make_identity

F32 = mybir.dt.float32
F32R = mybir.dt.float32r
BF16 = mybir.dt.bfloat16
AF = mybir.ActivationFunctionType
AX = mybir.AxisListType
MAX = mybir.AluOpType.max

DIM = 1024
HEADS = 16
DH = 64
FF = 4096
EPS = 1e-5
T = 2048        # tokens per batch (per-core KV scope)
NQ = 512        # own query tokens per core
KD = DIM // 128   # 8 feature tiles
CHUNK = 512
NCHUNK = T // CHUNK  # 4
N_CORES = 8
VW = DH + 1     # V columns per head incl. ones column

_cache = {}


def _quantize(w):
    w = w.astype(np.float32)
    return np.round(np.clip(w, -2.0, 2.0) * np.float32(0.75) + np.float32(0.5)) - np.float32(0.5)


def _prep_weights(i):
    """Host-side: quantize, fold scales/LN-params, transpose to [in, out]."""
    q = {k: _quantize(i[k]) for k in ("wq", "wk", "wv", "wo", "w1", "w2")}
    Wq = q["wq"] * i["sq"][:, None]
    Wk = q["wk"] * i["sk"][:, None]
    Wv = q["wv"] * i["sv"][:, None]
    Wo = q["wo"] * i["so"][:, None]
    W1 = q["w1"] * i["s1"][:, None]
    W2 = q["w2"] * i["s2"][:, None]
    g1, b1n = i["ln1_g"], i["ln1_b"]
    g2, b2n = i["ln2_g"], i["ln2_b"]
    s8 = np.float32(DH ** -0.5)
    out = {}
    out["wqT"] = np.ascontiguousarray((Wq * g1[None, :] * s8).T.astype(np.float32))
    out["bq"] = ((Wq @ b1n + i["bq"]) * s8).astype(np.float32)
    out["wkT"] = np.ascontiguousarray((Wk * g1[None, :]).T.astype(np.float32))
    out["bk"] = (Wk @ b1n + i["bk"]).astype(np.float32)
    out["wvT"] = np.ascontiguousarray((Wv * g1[None, :]).T.astype(ml_dtypes.bfloat16))
    bv = Wv @ b1n + i["bv"]
    out["woT"] = np.ascontiguousarray(Wo.T.astype(ml_dtypes.bfloat16))
    out["bo"] = (Wo @ bv + i["bo"]).astype(np.float32)
    out["w1T"] = np.ascontiguousarray((W1 * g2[None, :]).T.astype(ml_dtypes.bfloat16))
    out["b1"] = (W1 @ b2n + i["b1"]).astype(np.float32)
    out["w2T"] = np.ascontiguousarray(W2.T.astype(ml_dtypes.bfloat16))
    out["b2"] = i["b2"].astype(np.float32)
    return out


def _wslice(d, name, m, mm=128):
    """[DIM_in, n_out] weight dram -> lhsT tile view [128, KD_in, mm] for
    out-block m."""
    return d[name].rearrange("(k p) (mb mm) -> p k mb mm", p=128, mm=mm)[:, :, m]


def _build(zero_bias):
    nc = bacc.Bacc("TRN2", target_bir_lowering=False, debug=False,
                   num_devices=N_CORES)
    d = {}
    d["xT"] = nc.dram_tensor("xT", [DIM, T], F32R, kind="ExternalInput").ap()
    d["wqT"] = nc.dram_tensor("wqT", [DIM, DIM], F32R, kind="ExternalInput").ap()
    d["wkT"] = nc.dram_tensor("wkT", [DIM, DIM], F32R, kind="ExternalInput").ap()
    d["wvT"] = nc.dram_tensor("wvT", [DIM, DIM], BF16, kind="ExternalInput").ap()
    d["woT"] = nc.dram_tensor("woT", [DIM, DIM], BF16, kind="ExternalInput").ap()
    d["w1T"] = nc.dram_tensor("w1T", [DIM, FF], BF16, kind="ExternalInput").ap()
    d["w2T"] = nc.dram_tensor("w2T", [FF, DIM], BF16, kind="ExternalInput").ap()
    for nm, n in [("bq", DIM), ("bk", DIM), ("bo", DIM), ("b1", FF), ("b2", DIM)]:
        d[nm] = nc.dram_tensor(nm, [n], F32, kind="ExternalInput").ap()
    d["yT"] = nc.dram_tensor("yT", [DIM, NQ], F32, kind="ExternalOutput").ap()
    with tile.TileContext(nc) as tc:
        _body(nc, tc, d, zero_bias)
    nc.compile()
    return nc


def _body(nc, tc, d, zero_bias):
    ctx = ExitStack()
    with ctx:
        const = ctx.enter_context(tc.tile_pool(name="const", bufs=1))
        ones_blk = const.tile([128, 128], F32, name="ones_blk")
        nc.vector.memset(ones_blk[:], 1.0)
        ones1 = ones_blk[0:1, :]
        ones = const.tile([128, 1], F32R, name="ones")
        nc.vector.tensor_copy(ones[:], ones_blk[:, 0:1])
        bias = {}
        if not zero_bias:
            for nm, n in [("bq", DIM), ("bk", DIM), ("bo", DIM), ("b1", FF),
                          ("b2", DIM)]:
                t = const.tile([128, n // 128], F32, name=f"sb_{nm}")
                nc.sync.dma_start(out=t[:], in_=d[nm].rearrange("(m p) -> p m", p=128))
                bias[nm] = t

        # outputs of phase B that phase C consumes. Head h's softmax
        # denominator row lives at the 32-aligned partition 32*(h%4) of
        # DEN_tiles[h//4] (single-partition writes must be 32-aligned).
        ATTN_mbs = [const.tile([128, NQ], BF16, name=f"ATTN_{i}") for i in range(KD)]
        DEN_tiles = [const.tile([128, NQ], F32R, name=f"DEN_{i}") for i in range(4)]
        for t in DEN_tiles:
            nc.vector.memset(t[:].bitcast(F32), 1.0)

        xT_t = d["xT"].rearrange("(k p) t -> p k t", p=128)

        abctx = ExitStack()
        persist = abctx.enter_context(tc.tile_pool(name="persist", bufs=1))
        # long-lived activations (phases A+B only; freed before phase C)
        KT_sb = persist.tile([128, KD, T], F32R, name="KT_sb")          # 8MB
        V_sb = persist.tile([128, T // 128, 16 * VW], BF16, name="V_sb")  # 4.06MB
        QT_sb = persist.tile([128, KD, NQ], F32R, name="QT_sb")         # 2MB

        # ones columns of V (col 64 of each head block)
        vview = V_sb.rearrange("p b (h c) -> p b h c", c=VW)
        nc.vector.memset(vview[:, :, :, DH:DH + 1], 1.0)

        def act_store(dst, src, bname, m):
            """PSUM->SBUF move with optional bias."""
            if zero_bias:
                nc.scalar.copy(dst, src)
            else:
                nc.scalar.activation(dst, src, AF.Identity,
                                     bias=bias[bname][:, m:m + 1])

        # ============ Phase A: LN1 + K/V proj; Q proj on chunk 0 ============
        with ExitStack() as actx:
            sb_ln = actx.enter_context(tc.tile_pool(name="sb_ln", bufs=1))
            ps_bc = actx.enter_context(tc.tile_pool(name="ps_bc", bufs=1, space="PSUM"))
            sb_xt = actx.enter_context(tc.tile_pool(name="sb_xt", bufs=2))
            sb_sq = actx.enter_context(tc.tile_pool(name="sb_sq", bufs=1))
            sb_xhbf = actx.enter_context(tc.tile_pool(name="sb_xhbf", bufs=2))
            wstr = actx.enter_context(tc.tile_pool(name="wstr", bufs=3))
            wstrv = actx.enter_context(tc.tile_pool(name="wstrv", bufs=2))
            ps_stat = actx.enter_context(tc.tile_pool(name="ps_stat", bufs=1, space="PSUM"))
            ps_mm = actx.enter_context(tc.tile_pool(name="ps_mm", bufs=4, space="PSUM"))

            xts = [None] * NCHUNK
            rbs = [None] * NCHUNK

            def emit_dma(c):
                # issued from the (otherwise idle) gpsimd queue so waits on
                # the xt ring don't block the weight-stream queue
                xt = sb_xt.tile([128, KD, CHUNK], F32R, name="xt")
                nc.gpsimd.dma_start(out=xt[:], in_=xT_t[:, :, c * CHUNK:(c + 1) * CHUNK])
                xts[c] = xt

            def emit_stats(c):
                """LN stats for chunk c + broadcast tiles (PE + small DVE)."""
                xt = xts[c]
                ssum = ps_stat.tile([1, CHUNK], F32, name="ssum", tag="ssum")
                ssq = ps_stat.tile([1, CHUNK], F32, name="ssq", tag="ssq")
                for k in range(KD):
                    sq = sb_sq.tile([128, CHUNK], F32R, name="sq", tag="sq")
                    nc.scalar.activation(sq[:], xt[:, k], AF.Square)
                    nc.tensor.matmul(ssum[:], lhsT=ones[:], rhs=xt[:, k],
                                     start=(k == 0), stop=(k == KD - 1))
                    nc.tensor.matmul(ssq[:], lhsT=ones[:], rhs=sq[:],
                                     start=(k == 0), stop=(k == KD - 1))
                mu = sb_ln.tile([1, CHUNK], F32, name="mu", tag="mu")
                nc.vector.tensor_scalar_mul(mu[:], ssum[:], 1.0 / DIM)
                ta = sb_ln.tile([1, CHUNK], F32, name="ta", tag="ta")
                tb = sb_ln.tile([1, CHUNK], F32, name="tb", tag="tb")
                nc.vector.tensor_mul(ta[:], mu[:], mu[:])
                nc.vector.tensor_scalar(tb[:], ssq[:], 1.0 / DIM, None,
                                        mybir.AluOpType.mult)
                nc.vector.tensor_sub(tb[:], tb[:], ta[:])
                nc.vector.tensor_scalar_add(tb[:], tb[:], float(EPS))
                nc.scalar.activation(ta[:], tb[:], AF.Sqrt)
                nc.vector.reciprocal(tb[:], ta[:])
                mu_b = ps_bc.tile([128, CHUNK], F32, name="mu_b", tag="mu_b")
                r_b = ps_bc.tile([128, CHUNK], F32, name="r_b", tag="r_b")
                nc.tensor.matmul(mu_b[:], lhsT=ones1[:], rhs=mu[:], start=True, stop=True)
                nc.tensor.matmul(r_b[:], lhsT=ones1[:], rhs=tb[:], start=True, stop=True)
                rbs[c] = (mu_b, r_b)

            def emit_norm(c):
                """In-place normalize chunk c: xt <- (xt - mu) * r, plus bf16
                cast for the V-proj stationary side (DVE + Act)."""
                xt = xts[c]
                mu_b, r_b = rbs[c]
                xh_bf = sb_xhbf.tile([128, KD, CHUNK], BF16, name="xh_bf")
                for k in range(KD):
                    nc.vector.tensor_sub(xt[:, k], xt[:, k], mu_b[:])
                    nc.vector.tensor_mul(xt[:, k], xt[:, k], r_b[:])
                    nc.scalar.copy(xh_bf[:, k], xt[:, k])
                return xh_bf

            def emit_kpair(pair):
                """K proj for chunk pair: each streamed weight block serves
                both chunks (halves weight traffic, 2x DMA pacing slack)."""
                cs = (2 * pair, 2 * pair + 1)
                for m in range(KD):
                    wk = wstr.tile([128, KD, 128], F32R, name="wk", tag="wk")
                    nc.sync.dma_start(out=wk[:], in_=_wslice(d, "wkT", m))
                    for c in cs:
                        kp = ps_mm.tile([128, CHUNK], F32, name="kp", tag="mm")
                        for k in range(KD):
                            nc.tensor.matmul(kp[:], lhsT=wk[:, k], rhs=xts[c][:, k],
                                             start=(k == 0), stop=(k == KD - 1))
                        act_store(KT_sb[:, m, c * CHUNK:(c + 1) * CHUNK], kp[:],
                                  "bk", m)

            def emit_vpair(pair, xbs):
                wvT_v = d["wvT"].rearrange("(kh k p) (nb nn) -> p kh k nb nn",
                                           p=128, k=4, nn=CHUNK)
                cs = (2 * pair, 2 * pair + 1)
                for nb in range(2):
                    wvs = []
                    for kh in range(2):
                        wv = wstrv.tile([128, 4, CHUNK], BF16, name="wv", tag="wv")
                        nc.sync.dma_start(out=wv[:], in_=wvT_v[:, kh, :, nb])
                        wvs.append(wv)
                    for ci, c in enumerate(cs):
                        for t_sub in range(CHUNK // 128):
                            blk = c * 4 + t_sub
                            vp = ps_mm.tile([128, CHUNK], F32, name="vp", tag="mm")
                            for k in range(KD):
                                nc.tensor.matmul(
                                    vp[:],
                                    lhsT=xbs[ci][:, k, t_sub * 128:(t_sub + 1) * 128],
                                    rhs=wvs[k // 4][:, k % 4], start=(k == 0),
                                    stop=(k == KD - 1))
                            dst = vview[:, blk, 8 * nb:8 * nb + 8, 0:DH]
                            nc.scalar.copy(dst,
                                           vp.rearrange("p (h c) -> p h c", c=DH))

            def emit_qproj():
                xt = xts[0]
                for m in range(KD):
                    wq = wstr.tile([128, KD, 128], F32R, name="wq", tag="wk")
                    nc.sync.dma_start(out=wq[:], in_=_wslice(d, "wqT", m))
                    qp = ps_mm.tile([128, CHUNK], F32, name="qp", tag="mm")
                    for k in range(KD):
                        nc.tensor.matmul(qp[:], lhsT=wq[:, k], rhs=xt[:, k],
                                         start=(k == 0), stop=(k == KD - 1))
                    act_store(QT_sb[:, m], qp[:], "bq", m)

            # pipelined emission (Q right after K-pair 0 so xt(0) frees for
            # the chunk-2 DMA without stalling the PE queue)
            emit_dma(0)
            emit_dma(1)
            emit_stats(0)
            xb0 = emit_norm(0)
            emit_stats(1)
            xb1 = emit_norm(1)
            emit_kpair(0)
            emit_qproj()
            emit_dma(2)
            emit_dma(3)
            emit_stats(2)
            emit_stats(3)
            emit_vpair(0, (xb0, xb1))
            xb2 = emit_norm(2)
            xb3 = emit_norm(3)
            emit_kpair(1)
            emit_vpair(1, (xb2, xb3))

        # ============ Phase B: attention ============
        # A^T is produced by the DMA crossbar (dma_start transpose=True),
        # not the PE: no PE transposes, no PSUM->SBUF staging copies. S
        # streams through a 7-deep ring of single-bank PSUM tiles; AV runs
        # one head behind with 512-wide moving ops; V's ones column makes
        # av row 64 the softmax denominator.
        with ExitStack() as bctx:
            ps_S = bctx.enter_context(tc.tile_pool(name="ps_S", bufs=7, space="PSUM"))
            ps_av = bctx.enter_context(tc.tile_pool(name="ps_av", bufs=1, space="PSUM"))
            sb_A = bctx.enter_context(tc.tile_pool(name="sb_A", bufs=4))
            sb_AT = bctx.enter_context(tc.tile_pool(name="sb_AT", bufs=2))
            sb_sm = bctx.enter_context(tc.tile_pool(name="sb_sm", bufs=8))

            state = {}  # per-head in-flight tiles

            def emit_qk_softmax(h, qt):
                """S chunks for (h, qt); per-chunk max + combine; exp -> A;
                XBAR transpose of A into AT[:, :, qt, :]."""
                mb, r0 = h // 2, (h % 2) * 64
                q_sl = QT_sb[r0:r0 + 64, mb, qt * 128:(qt + 1) * 128]
                if qt == 0:
                    state[h] = {"AT": sb_AT.tile([128, T // 128, 4, 128], BF16,
                                                 name="AT")}
                A = sb_A.tile([128, T], BF16, name="A")
                Ss = []
                for c in range(NCHUNK):
                    S = ps_S.tile([128, CHUNK], F32, name="S")
                    nc.tensor.matmul(
                        S[:], lhsT=q_sl,
                        rhs=KT_sb[r0:r0 + 64, mb, c * CHUNK:(c + 1) * CHUNK],
                        start=True, stop=True)
                    Ss.append(S)
                ms = []
                for c in range(NCHUNK):
                    m = sb_sm.tile([128, 1], F32, name=f"m{c}", tag=f"m{c}")
                    nc.vector.reduce_max(m[:], Ss[c][:], axis=AX.X)
                    ms.append(m)
                m01 = sb_sm.tile([128, 1], F32, name="m01", tag="m01")
                m23 = sb_sm.tile([128, 1], F32, name="m23", tag="m23")
                negm = sb_sm.tile([128, 1], F32, name="negm", tag="negm")
                nc.vector.tensor_tensor(m01[:], ms[0][:], ms[1][:], MAX)
                nc.vector.tensor_tensor(m23[:], ms[2][:], ms[3][:], MAX)
                nc.vector.tensor_tensor(negm[:], m01[:], m23[:], MAX)
                nc.vector.tensor_scalar_mul(negm[:], negm[:], -1.0)
                for c in range(NCHUNK):
                    nc.scalar.activation(A[:, c * CHUNK:(c + 1) * CHUNK],
                                         Ss[c][:], AF.Exp, bias=negm[:])
                nc.sync.dma_start(out=state[h]["AT"][:, :, qt, :], in_=A[:],
                                  transpose=True)

            def emit_av(h):
                """AV for head h (all queries, 512-wide moving); av row 64 is
                the denominator (V ones column)."""
                mb, r0 = h // 2, (h % 2) * 64
                AT = state[h]["AT"]
                av = ps_av.tile([VW, NQ], F32, name="av")
                for kb in range(T // 128):
                    nc.tensor.matmul(av[:], lhsT=V_sb[:, kb, VW * h:VW * (h + 1)],
                                     rhs=AT[:, kb], start=(kb == 0),
                                     stop=(kb == T // 128 - 1))
                nc.scalar.copy(ATTN_mbs[mb][r0:r0 + 64, :], av[0:DH])
                nc.vector.tensor_copy(
                    DEN_tiles[h // 4][32 * (h % 4):32 * (h % 4) + 1, :],
                    av[DH:DH + 1])
                state[h] = None

            for h in range(HEADS):
                for qt in range(NQ // 128):
                    if qt == 1 and h >= 1:
                        emit_av(h - 1)
                    emit_qk_softmax(h, qt)
            emit_av(HEADS - 1)

        abctx.close()  # free KT/V/QT SBUF before phase C pools open

        # ============ Phase C: normalize + O proj + residual + LN2 + FF ====
        with ExitStack() as cctx:
            sb_ln2 = cctx.enter_context(tc.tile_pool(name="sb_ln2", bufs=1))
            ps_bc2 = cctx.enter_context(tc.tile_pool(name="ps_bc2", bufs=2, space="PSUM"))
            scr2 = cctx.enter_context(tc.tile_pool(name="scr2", bufs=2))
            sb_u = cctx.enter_context(tc.tile_pool(name="sb_u", bufs=1))
            sb_attn = cctx.enter_context(tc.tile_pool(name="sb_attn", bufs=1))
            wstr2 = cctx.enter_context(tc.tile_pool(name="wstr2", bufs=3))
            ps_stat2 = cctx.enter_context(tc.tile_pool(name="ps_stat2", bufs=1, space="PSUM"))
            ps_mm2 = cctx.enter_context(tc.tile_pool(name="ps_mm2", bufs=4, space="PSUM"))
            sb_xr = cctx.enter_context(tc.tile_pool(name="sb_xr", bufs=2))

            # --- batched attention normalize ---
            # DEN tiles -> reciprocal in place; selector matmuls broadcast
            # rden rows (at partitions 0/32/64/96) to the 64-row head halves.
            with nc.allow_low_precision(reason="f32r is fp32-width; only "
                                        "tagged f32r for fast selector matmul"):
                for t in DEN_tiles:
                    nc.vector.reciprocal(t[:], t[:])
            rsels = []
            for i in range(2):
                rs = sb_ln2.tile([128, 128], F32R, name=f"rsel{i}")
                nc.vector.memset(rs[:].bitcast(F32), 0.0)
                base = 64 * i
                nc.vector.memset(rs[base:base + 1, 0:64].bitcast(F32), 1.0)
                nc.vector.memset(rs[base + 32:base + 33, 64:128].bitcast(F32), 1.0)
                rsels.append(rs)
            attn_n = sb_attn.tile([128, KD, NQ], BF16, name="attn_n")
            for mb in range(KD):
                rb = ps_bc2.tile([128, NQ], F32, name="rb", tag="rb")
                nc.tensor.matmul(rb[:], lhsT=rsels[mb % 2][:],
                                 rhs=DEN_tiles[mb // 2][:], start=True, stop=True)
                nc.vector.tensor_mul(attn_n[:, mb], ATTN_mbs[mb][:], rb[:])

            # --- O proj + residual -> u ---
            u_sb = sb_u.tile([128, KD, NQ], F32R, name="u_sb")
            for m in range(KD):
                wo = wstr2.tile([128, KD, 128], BF16, name="wo", tag="wsm")
                nc.sync.dma_start(out=wo[:], in_=_wslice(d, "woT", m))
                op = ps_mm2.tile([128, NQ], F32, name="op", tag="mm")
                for k in range(KD):
                    nc.tensor.matmul(op[:], lhsT=wo[:, k], rhs=attn_n[:, k],
                                     start=(k == 0), stop=(k == KD - 1))
                xr = sb_xr.tile([128, NQ], F32R, name="xr", tag="xr")
                nc.sync.dma_start(out=xr[:], in_=xT_t[:, m, 0:NQ])
                if zero_bias:
                    nc.vector.tensor_add(u_sb[:, m], op[:], xr[:])
                else:
                    upre = scr2.tile([128, NQ], F32, name="upre", tag="scr")
                    nc.vector.tensor_add(upre[:], op[:], xr[:])
                    nc.scalar.activation(u_sb[:, m], upre[:], AF.Identity,
                                         bias=bias["bo"][:, m:m + 1])

            # --- LN2 (classic, from u_sb into uh bf16) ---
            ssum = ps_stat2.tile([1, NQ], F32, name="ssum2")
            ssq = ps_stat2.tile([1, NQ], F32, name="ssq2")
            for k in range(KD):
                sq = scr2.tile([128, NQ], F32R, name="sq2", tag="scr")
                nc.scalar.activation(sq[:], u_sb[:, k], AF.Square)
                nc.tensor.matmul(ssum[:], lhsT=ones[:], rhs=u_sb[:, k],
                                 start=(k == 0), stop=(k == KD - 1))
                nc.tensor.matmul(ssq[:], lhsT=ones[:], rhs=sq[:],
                                 start=(k == 0), stop=(k == KD - 1))
            mu = sb_ln2.tile([1, NQ], F32, name="mu2")
            nc.vector.tensor_scalar_mul(mu[:], ssum[:], 1.0 / DIM)
            var = sb_ln2.tile([1, NQ], F32, name="var2")
            musq = sb_ln2.tile([1, NQ], F32, name="musq2")
            nc.vector.tensor_mul(musq[:], mu[:], mu[:])
            nc.vector.tensor_scalar(var[:], ssq[:], 1.0 / DIM, None,
                                    mybir.AluOpType.mult)
            nc.vector.tensor_sub(var[:], var[:], musq[:])
            nc.vector.tensor_scalar_add(var[:], var[:], float(EPS))
            sd = sb_ln2.tile([1, NQ], F32, name="sd2")
            nc.scalar.activation(sd[:], var[:], AF.Sqrt)
            r = sb_ln2.tile([1, NQ], F32, name="r2")
            nc.vector.reciprocal(r[:], sd[:])
            mu_b = ps_bc2.tile([128, NQ], F32, name="mu_b2", tag="rb")
            r_b = ps_bc2.tile([128, NQ], F32, name="r_b2", tag="rb")
            nc.tensor.matmul(mu_b[:], lhsT=ones1[:], rhs=mu[:], start=True, stop=True)
            nc.tensor.matmul(r_b[:], lhsT=ones1[:], rhs=r[:], start=True, stop=True)
            uh_bf = sb_attn.tile([128, KD, NQ], BF16, name="uh_bf")
            for k in range(KD):
                xc = scr2.tile([128, NQ], F32, name="uhc", tag="scr")
                nc.vector.tensor_sub(xc[:], u_sb[:, k], mu_b[:])
                nc.vector.tensor_mul(uh_bf[:, k], xc[:], r_b[:])

            # --- FF1 + GELU ---
            H_sb = sb_u.tile([128, FF // 128, NQ], BF16, name="H_sb")
            for m in range(FF // 128):
                w1 = wstr2.tile([128, KD, 128], BF16, name="w1", tag="wsm")
                nc.sync.dma_start(out=w1[:], in_=_wslice(d, "w1T", m))
                h1 = ps_mm2.tile([128, NQ], F32, name="h1", tag="mm")
                for k in range(KD):
                    nc.tensor.matmul(h1[:], lhsT=w1[:, k], rhs=uh_bf[:, k],
                                     start=(k == 0), stop=(k == KD - 1))
                if zero_bias:
                    nc.scalar.activation(H_sb[:, m], h1[:], AF.Gelu)
                else:
                    nc.scalar.activation(H_sb[:, m], h1[:], AF.Gelu,
                                         bias=bias["b1"][:, m:m + 1])

            # --- FF2 + residual + out ---
            w2T_v = d["w2T"].rearrange("(kh k p) (mb mm) -> p kh k mb mm",
                                       p=128, k=8, mm=128)
            for m in range(KD):
                f2 = ps_mm2.tile([128, NQ], F32, name="f2", tag="mm")
                for kh in range(4):
                    w2 = wstr2.tile([128, 8, 128], BF16, name="w2", tag="w2")
                    nc.sync.dma_start(out=w2[:], in_=w2T_v[:, kh, :, m])
                    for k in range(8):
                        nc.tensor.matmul(f2[:], lhsT=w2[:, k], rhs=H_sb[:, kh * 8 + k],
                                         start=(kh == 0 and k == 0),
                                         stop=(kh == 3 and k == 7))
                oout = scr2.tile([128, NQ], F32, name="oout", tag="scr")
                nc.vector.tensor_add(oout[:], f2[:], u_sb[:, m])
                if not zero_bias:
                    nc.scalar.activation(oout[:], oout[:], AF.Identity,
                                         bias=bias["b2"][:, m:m + 1])
                nc.sync.dma_start(out=d["yT"][m * 128:(m + 1) * 128, :], in_=oout[:])


def kernel(**inputs) -> np.ndarray:
    inputs = {k: np.asarray(v) for k, v in inputs.items()}
    x = inputs["x"].astype(np.float32)
    B, N, D = x.shape  # (2, 2048, 1024)
    w = _prep_weights(inputs)

    zero_bias = all(not np.any(w[nm]) for nm in ("bq", "bk", "bo", "b1", "b2"))
    key = ("nc", zero_bias)
    if key not in _cache:
        _cache[key] = _build(zero_bias)
    nc = _cache[key]
    _cache["nc"] = nc

    per_batch = N_CORES // B  # 4
    in_maps = []
    for c in range(N_CORES):
        b, chunk = divmod(c, per_batch)
        xT = np.ascontiguousarray(np.roll(x[b].T, -chunk * NQ, axis=1))
        m = {"xT": xT}
        m.update(w)
        in_maps.append(m)
    res = run_bass_kernel_spmd(nc, in_maps, core_ids=list(range(N_CORES)))
    out = np.empty((B, N, D), dtype=np.float32)
    for c in range(N_CORES):
        b, chunk = divmod(c, per_batch)
        out[b, chunk * NQ:(chunk + 1) * NQ, :] = res.results[c]["yT"].T
    return out


# revision 30
# speedup vs baseline: 1.0729x; 1.0729x over previous
"""BitNet transformer block kernel for 8 Trainium2 NeuronCores.

Sharding: data-parallel. Core c handles batch c//4, token chunk c%4 (512
query tokens). Each core computes K/V for its batch's full 2048-token
sequence (redundant KV compute instead of collectives). Host rotates the
token axis per core so every core's own tokens sit at chunk 0 -> all 8
cores run one identical SPMD program.

Layout: activations feature-major (x^T tiles [128 feat, T tok]) so matmul
contraction (features) lies on partitions. V is token-major for the AV
matmul. Weights host-prepped: quantized, scales/LN-gamma folded in; biases
folded/propagated. Q/K/logit path in float32r (TF32-like, ~1e-4 rel),
everything else bf16 with fp32 accumulation.
"""
import sys

sys.path.insert(0, "/opt/trn_rl_repo")

import numpy as np
import ml_dtypes
from contextlib import ExitStack

import concourse.bass as bass
import concourse.bacc as bacc
import concourse.tile as tile
from concourse import mybir
from concourse.bass_utils import run_bass_kernel_spmd
from concourse.masks import make_identity

F32 = mybir.dt.float32
F32R = mybir.dt.float32r
BF16 = mybir.dt.bfloat16
AF = mybir.ActivationFunctionType
AX = mybir.AxisListType

DIM = 1024
HEADS = 16
DH = 64
FF = 4096
EPS = 1e-5
T = 2048        # tokens per batch (per-core KV scope)
NQ = 512        # own query tokens per core
KD = DIM // 128   # 8 feature tiles
CHUNK = 512
NCHUNK = T // CHUNK  # 4
N_CORES = 8

_cache = {}


def _quantize(w):
    w = w.astype(np.float32)
    return np.round(np.clip(w, -2.0, 2.0) * np.float32(0.75) + np.float32(0.5)) - np.float32(0.5)


def _make_e4():
    e4 = np.zeros((4, 256), np.float32)
    for qt in range(4):
        e4[qt, qt * 64:(qt + 1) * 64] = 1.0
    return e4


def _prep_weights(i):
    """Host-side: quantize, fold scales/LN-params, transpose to [in, out]."""
    q = {k: _quantize(i[k]) for k in ("wq", "wk", "wv", "wo", "w1", "w2")}
    Wq = q["wq"] * i["sq"][:, None]
    Wk = q["wk"] * i["sk"][:, None]
    Wv = q["wv"] * i["sv"][:, None]
    Wo = q["wo"] * i["so"][:, None]
    W1 = q["w1"] * i["s1"][:, None]
    W2 = q["w2"] * i["s2"][:, None]
    g1, b1n = i["ln1_g"], i["ln1_b"]
    g2, b2n = i["ln2_g"], i["ln2_b"]
    s8 = np.float32(DH ** -0.5)
    out = {}
    out["wqT"] = np.ascontiguousarray((Wq * g1[None, :] * s8).T.astype(np.float32))
    out["bq"] = ((Wq @ b1n + i["bq"]) * s8).astype(np.float32)
    out["wkT"] = np.ascontiguousarray((Wk * g1[None, :]).T.astype(np.float32))
    out["bk"] = (Wk @ b1n + i["bk"]).astype(np.float32)
    out["wvT"] = np.ascontiguousarray((Wv * g1[None, :]).T.astype(ml_dtypes.bfloat16))
    bv = Wv @ b1n + i["bv"]
    out["woT"] = np.ascontiguousarray(Wo.T.astype(ml_dtypes.bfloat16))
    out["bo"] = (Wo @ bv + i["bo"]).astype(np.float32)
    out["w1T"] = np.ascontiguousarray((W1 * g2[None, :]).T.astype(ml_dtypes.bfloat16))
    out["b1"] = (W1 @ b2n + i["b1"]).astype(np.float32)
    out["w2T"] = np.ascontiguousarray(W2.T.astype(ml_dtypes.bfloat16))
    out["b2"] = i["b2"].astype(np.float32)
    out["e4"] = _make_e4()
    return out


def _bcast_ap(t):
    """(unused) Partition-broadcast read AP of a [1, N] sbuf tile."""
    return bass.AP(tensor=t.tensor, offset=t.offset,
                   ap=[[0, 128]] + [list(a) for a in t.ap[1:]])


def _ln_chunk(nc, sb, ps_bc, scratch, ps_stat, xh_pool, xt, ones, ones1, width,
              out_dt=F32R):
    """LayerNorm transform of one feature-major chunk [128, KD, width].
    Returns xh = (x - mu) * rstd."""
    ssum = ps_stat.tile([1, width], F32, name="ssum")
    ssq = ps_stat.tile([1, width], F32, name="ssq")
    for k in range(KD):
        sq = scratch.tile([128, width], F32R, name="scr", tag="scr")
        nc.scalar.activation(sq[:], xt[:, k], AF.Square)
        nc.tensor.matmul(ssum[:], lhsT=ones[:], rhs=xt[:, k],
                         start=(k == 0), stop=(k == KD - 1))
        nc.tensor.matmul(ssq[:], lhsT=ones[:], rhs=sq[:],
                         start=(k == 0), stop=(k == KD - 1))
    mu = sb.tile([1, width], F32, name="mu")
    nc.vector.tensor_scalar_mul(mu[:], ssum[:], 1.0 / DIM)
    var = sb.tile([1, width], F32, name="var")
    # var = ssq/DIM - mu^2  via (ssq*(1/DIM) - mu*mu)
    musq = sb.tile([1, width], F32, name="musq")
    nc.vector.tensor_mul(musq[:], mu[:], mu[:])
    nc.vector.tensor_scalar(var[:], ssq[:], 1.0 / DIM, None,
                            mybir.AluOpType.mult)
    nc.vector.tensor_sub(var[:], var[:], musq[:])
    nc.vector.tensor_scalar_add(var[:], var[:], float(EPS))
    sd = sb.tile([1, width], F32, name="sd")
    nc.scalar.activation(sd[:], var[:], AF.Sqrt)
    r = sb.tile([1, width], F32, name="r")
    nc.vector.reciprocal(r[:], sd[:])
    mu_b = ps_bc.tile([128, width], F32, name="mu_b")
    r_b = ps_bc.tile([128, width], F32, name="r_b")
    nc.tensor.matmul(mu_b[:], lhsT=ones1[:], rhs=mu[:], start=True, stop=True)
    nc.tensor.matmul(r_b[:], lhsT=ones1[:], rhs=r[:], start=True, stop=True)
    xh = xh_pool.tile([128, KD, width], out_dt, name="xh")
    for k in range(KD):
        xc = scratch.tile([128, width], F32, name="scr2", tag="scr")
        nc.vector.tensor_sub(xc[:], xt[:, k], mu_b[:])
        nc.vector.tensor_mul(xh[:, k], xc[:], r_b[:])
    return xh


def _wslice(d, name, m, mm=128):
    """[DIM_in, n_out] weight dram -> lhsT tile view [128, KD_in, mm] for
    out-block m."""
    return d[name].rearrange("(k p) (mb mm) -> p k mb mm", p=128, mm=mm)[:, :, m]


def _body(nc, tc, d):
    ctx = ExitStack()
    with ctx:
        const = ctx.enter_context(tc.tile_pool(name="const", bufs=1))
        ones_blk = const.tile([128, 128], F32, name="ones_blk")
        nc.vector.memset(ones_blk[:], 1.0)
        ones1 = ones_blk[0:1, :]
        ones = const.tile([128, 1], F32R, name="ones")
        nc.vector.tensor_copy(ones[:], ones_blk[:, 0:1])
        ident = const.tile([128, 128], BF16, name="ident")
        make_identity(nc, ident)

        bias = {}
        for nm, n in [("bq", DIM), ("bk", DIM), ("bo", DIM), ("b1", FF), ("b2", DIM)]:
            t = const.tile([128, n // 128], F32, name=f"sb_{nm}")
            nc.sync.dma_start(out=t[:], in_=d[nm].rearrange("(m p) -> p m", p=128))
            bias[nm] = t

        # long-lived activations
        KT_sb = const.tile([128, KD, T], F32R, name="KT_sb")        # 8MB
        V_sb = const.tile([128, T // 128, DIM], BF16, name="V_sb")  # 4MB
        QT_sb = const.tile([128, KD, NQ], F32R, name="QT_sb")       # 2MB
        ATTN_mbs = [const.tile([128, NQ], BF16, name=f"ATTN_{i}") for i in range(KD)]

        xT_t = d["xT"].rearrange("(k p) t -> p k t", p=128)

        # ---- Phase A: LN1 + K/V proj per chunk; Q proj on chunk 0 ----
        with ExitStack() as actx:
            sb_ln = actx.enter_context(tc.tile_pool(name="sb_ln", bufs=1))
            ps_bc = actx.enter_context(tc.tile_pool(name="ps_bc", bufs=1, space="PSUM"))
            scratch = actx.enter_context(tc.tile_pool(name="scratch", bufs=2))
            sb_xt = actx.enter_context(tc.tile_pool(name="sb_xt", bufs=1))
            sb_xh = actx.enter_context(tc.tile_pool(name="sb_xh", bufs=2))
            sb_xhbf = actx.enter_context(tc.tile_pool(name="sb_xhbf", bufs=1))
            wstr = actx.enter_context(tc.tile_pool(name="wstr", bufs=2))
            ps_stat = actx.enter_context(tc.tile_pool(name="ps_stat", bufs=1, space="PSUM"))
            ps_mm = actx.enter_context(tc.tile_pool(name="ps_mm", bufs=4, space="PSUM"))

            for c in range(NCHUNK):
                xt = sb_xt.tile([128, KD, CHUNK], F32R, name="xt")
                nc.sync.dma_start(out=xt[:], in_=xT_t[:, :, c * CHUNK:(c + 1) * CHUNK])
                xh = _ln_chunk(nc, sb_ln, ps_bc, scratch, ps_stat, sb_xh, xt, ones, ones1, CHUNK)
                xh_bf = sb_xhbf.tile([128, KD, CHUNK], BF16, name="xh_bf")
                nc.vector.tensor_copy(xh_bf[:], xh[:])

                for m in range(KD):
                    wk = wstr.tile([128, KD, 128], F32R, name="wk", tag="wk")
                    nc.sync.dma_start(out=wk[:], in_=_wslice(d, "wkT", m))
                    kp = ps_mm.tile([128, CHUNK], F32, name="kp", tag="mm")
                    for k in range(KD):
                        nc.tensor.matmul(kp[:], lhsT=wk[:, k], rhs=xh[:, k],
                                         start=(k == 0), stop=(k == KD - 1))
                    nc.scalar.activation(KT_sb[:, m, c * CHUNK:(c + 1) * CHUNK], kp[:],
                                         AF.Identity, bias=bias["bk"][:, m:m + 1])
                wvT_v = d["wvT"].rearrange("(kh k p) (nb nn) -> p kh k nb nn",
                                           p=128, k=4, nn=CHUNK)
                for nb in range(2):
                    wvs = []
                    for kh in range(2):
                        wv = wstr.tile([128, 4, CHUNK], BF16, name="wv", tag="wv")
                        nc.sync.dma_start(out=wv[:], in_=wvT_v[:, kh, :, nb])
                        wvs.append(wv)
                    for t_sub in range(CHUNK // 128):
                        blk = c * 4 + t_sub
                        vp = ps_mm.tile([128, CHUNK], F32, name="vp", tag="mm")
                        for k in range(KD):
                            nc.tensor.matmul(
                                vp[:], lhsT=xh_bf[:, k, t_sub * 128:(t_sub + 1) * 128],
                                rhs=wvs[k // 4][:, k % 4], start=(k == 0), stop=(k == KD - 1))
                        nc.vector.tensor_copy(
                            V_sb[:, blk, nb * CHUNK:(nb + 1) * CHUNK], vp[:])
                if c == 0:
                    for m in range(KD):
                        wq = wstr.tile([128, KD, 128], F32R, name="wq", tag="wk")
                        nc.sync.dma_start(out=wq[:], in_=_wslice(d, "wqT", m))
                        qp = ps_mm.tile([128, CHUNK], F32, name="qp", tag="mm")
                        for k in range(KD):
                            nc.tensor.matmul(qp[:], lhsT=wq[:, k], rhs=xh[:, k],
                                             start=(k == 0), stop=(k == KD - 1))
                        nc.scalar.activation(QT_sb[:, m], qp[:],
                                             AF.Identity, bias=bias["bq"][:, m:m + 1])

        # ---- Phase B: attention ----
        # A^T is produced by the DMA crossbar (dma_start transpose=True)
        # instead of PE transposes + DVE copies; AV runs one head behind so
        # the XBAR latency hides under the next head's QK/softmax.
        with ExitStack() as bctx:
            sb_A = bctx.enter_context(tc.tile_pool(name="sb_A", bufs=6))
            sb_AT = bctx.enter_context(tc.tile_pool(name="sb_AT", bufs=2))
            sb_sm = bctx.enter_context(tc.tile_pool(name="sb_sm", bufs=4))
            ps_S = bctx.enter_context(tc.tile_pool(name="ps_S", bufs=3, space="PSUM"))
            sb_S = bctx.enter_context(tc.tile_pool(name="sb_S", bufs=3))
            ps_av = bctx.enter_context(tc.tile_pool(name="ps_av", bufs=1, space="PSUM"))

            ATs = {}

            def emit_av(h):
                mb, r0 = h // 2, (h % 2) * 64
                AT = ATs.pop(h)
                av = ps_av.tile([64, NQ], F32, name="av")
                for kb in range(T // 128):
                    nc.tensor.matmul(av[:], lhsT=V_sb[:, kb, h * DH:(h + 1) * DH],
                                     rhs=AT[:, kb],
                                     start=(kb == 0), stop=(kb == T // 128 - 1))
                nc.scalar.copy(ATTN_mbs[mb][r0:r0 + 64, :], av[:])

            for h in range(HEADS):
                mb, r0 = h // 2, (h % 2) * 64
                AT = sb_AT.tile([128, T // 128, NQ], BF16, name="AT")
                ATs[h] = AT
                for qt in range(NQ // 128):
                    if qt == 1 and h >= 1:
                        emit_av(h - 1)
                    A = sb_A.tile([128, T], BF16, name="A")
                    q_sl = QT_sb[r0:r0 + 64, mb, qt * 128:(qt + 1) * 128]
                    scp = sb_S.tile([128, 2, 2 * CHUNK], F32, name="scp")
                    for jj in range(2):
                        S = ps_S.tile([128, 2, CHUNK], F32, name="S")
                        for j in range(2):
                            k_off = (jj * 2 + j) * CHUNK
                            nc.tensor.matmul(
                                S[:, j], lhsT=q_sl,
                                rhs=KT_sb[r0:r0 + 64, mb, k_off:k_off + CHUNK],
                                start=True, stop=True)
                        nc.scalar.copy(scp[:, jj], S.rearrange("p a b -> p (a b)"))
                    scpf = scp.rearrange("p a b -> p (a b)")
                    negm = sb_sm.tile([128, 1], F32, name="negm")
                    nc.vector.reduce_max(negm[:], scpf[:], axis=AX.X)
                    nc.vector.tensor_scalar_mul(negm[:], negm[:], -1.0)
                    den = sb_sm.tile([128, 1], F32, name="den")
                    nc.scalar.activation(A[:], scpf[:], AF.Exp,
                                         bias=negm[:], accum_out=den[:])
                    rden = sb_sm.tile([128, 1], F32, name="rden")
                    nc.vector.reciprocal(rden[:], den[:])
                    nc.vector.tensor_scalar_mul(A[:], A[:], rden[:])
                    nc.sync.dma_start(out=AT[:, :, qt * 128:(qt + 1) * 128],
                                      in_=A[:], transpose=True)
            emit_av(HEADS - 1)

        # ---- Phase C: O proj + residual + LN2 + FF ----
        with ExitStack() as cctx:
            sb_ln2 = cctx.enter_context(tc.tile_pool(name="sb_ln2", bufs=1))
            ps_bc2 = cctx.enter_context(tc.tile_pool(name="ps_bc2", bufs=1, space="PSUM"))
            scr2 = cctx.enter_context(tc.tile_pool(name="scr2", bufs=2))
            sb_u = cctx.enter_context(tc.tile_pool(name="sb_u", bufs=1))
            wstr2 = cctx.enter_context(tc.tile_pool(name="wstr2", bufs=2))
            ps_stat2 = cctx.enter_context(tc.tile_pool(name="ps_stat2", bufs=1, space="PSUM"))
            ps_mm2 = cctx.enter_context(tc.tile_pool(name="ps_mm2", bufs=4, space="PSUM"))

            sb_xr = cctx.enter_context(tc.tile_pool(name="sb_xr", bufs=2))
            u_sb = sb_u.tile([128, KD, NQ], F32R, name="u_sb")
            for m in range(KD):
                wo = wstr2.tile([128, KD, 128], BF16, name="wo", tag="wsm")
                nc.sync.dma_start(out=wo[:], in_=_wslice(d, "woT", m))
                op = ps_mm2.tile([128, NQ], F32, name="op", tag="mm")
                for k in range(KD):
                    nc.tensor.matmul(op[:], lhsT=wo[:, k], rhs=ATTN_mbs[k][:],
                                     start=(k == 0), stop=(k == KD - 1))
                xr = sb_xr.tile([128, NQ], F32R, name="xr", tag="xr")
                nc.sync.dma_start(out=xr[:], in_=xT_t[:, m, 0:NQ])
                upre = scr2.tile([128, NQ], F32, name="upre", tag="scr")
                nc.vector.tensor_add(upre[:], op[:], xr[:])
                nc.scalar.activation(u_sb[:, m], upre[:], AF.Identity,
                                     bias=bias["bo"][:, m:m + 1])
            uh_bf = _ln_chunk(nc, sb_ln2, ps_bc2, scr2, ps_stat2, sb_u, u_sb, ones,
                              ones1, NQ, out_dt=BF16)
            H_sb = sb_u.tile([128, FF // 128, NQ], BF16, name="H_sb")
            for m in range(FF // 128):
                w1 = wstr2.tile([128, KD, 128], BF16, name="w1", tag="wsm")
                nc.sync.dma_start(out=w1[:], in_=_wslice(d, "w1T", m))
                h1 = ps_mm2.tile([128, NQ], F32, name="h1", tag="mm")
                for k in range(KD):
                    nc.tensor.matmul(h1[:], lhsT=w1[:, k], rhs=uh_bf[:, k],
                                     start=(k == 0), stop=(k == KD - 1))
                nc.scalar.activation(H_sb[:, m], h1[:], AF.Gelu,
                                     bias=bias["b1"][:, m:m + 1])
            w2T_v = d["w2T"].rearrange("(kh k p) (mb mm) -> p kh k mb mm",
                                       p=128, k=8, mm=128)
            for m in range(KD):
                f2 = ps_mm2.tile([128, NQ], F32, name="f2", tag="mm")
                for kh in range(4):
                    w2 = wstr2.tile([128, 8, 128], BF16, name="w2", tag="w2")
                    nc.sync.dma_start(out=w2[:], in_=w2T_v[:, kh, :, m])
                    for k in range(8):
                        nc.tensor.matmul(f2[:], lhsT=w2[:, k], rhs=H_sb[:, kh * 8 + k],
                                         start=(kh == 0 and k == 0),
                                         stop=(kh == 3 and k == 7))
                opre = scr2.tile([128, NQ], F32, name="opre", tag="scr")
                nc.vector.tensor_add(opre[:], f2[:], u_sb[:, m])
                oout = scr2.tile([128, NQ], F32, name="oout", tag="scr")
                nc.scalar.activation(oout[:], opre[:], AF.Identity,
                                     bias=bias["b2"][:, m:m + 1])
                nc.sync.dma_start(out=d["yT"][m * 128:(m + 1) * 128, :], in_=oout[:])


def _build():
    nc = bacc.Bacc("TRN2", target_bir_lowering=False, debug=False,
                   num_devices=N_CORES)
    d = {}
    d["xT"] = nc.dram_tensor("xT", [DIM, T], F32R, kind="ExternalInput").ap()
    d["wqT"] = nc.dram_tensor("wqT", [DIM, DIM], F32R, kind="ExternalInput").ap()
    d["wkT"] = nc.dram_tensor("wkT", [DIM, DIM], F32R, kind="ExternalInput").ap()
    d["wvT"] = nc.dram_tensor("wvT", [DIM, DIM], BF16, kind="ExternalInput").ap()
    d["woT"] = nc.dram_tensor("woT", [DIM, DIM], BF16, kind="ExternalInput").ap()
    d["w1T"] = nc.dram_tensor("w1T", [DIM, FF], BF16, kind="ExternalInput").ap()
    d["w2T"] = nc.dram_tensor("w2T", [FF, DIM], BF16, kind="ExternalInput").ap()
    for nm, n in [("bq", DIM), ("bk", DIM), ("bo", DIM), ("b1", FF), ("b2", DIM)]:
        d[nm] = nc.dram_tensor(nm, [n], F32, kind="ExternalInput").ap()
    d["e4"] = nc.dram_tensor("e4", [4, 256], F32, kind="ExternalInput").ap()
    d["yT"] = nc.dram_tensor("yT", [DIM, NQ], F32, kind="ExternalOutput").ap()
    with tile.TileContext(nc) as tc:
        _body(nc, tc, d)
    nc.compile()
    return nc


def kernel(**inputs) -> np.ndarray:
    inputs = {k: np.asarray(v) for k, v in inputs.items()}
    x = inputs["x"].astype(np.float32)
    B, N, D = x.shape  # (2, 2048, 1024)
    w = _prep_weights(inputs)

    if "nc" not in _cache:
        _cache["nc"] = _build()
    nc = _cache["nc"]

    per_batch = N_CORES // B  # 4
    in_maps = []
    for c in range(N_CORES):
        b, chunk = divmod(c, per_batch)
        xT = np.ascontiguousarray(np.roll(x[b].T, -chunk * NQ, axis=1))
        m = {"xT": xT}
        m.update(w)
        in_maps.append(m)
    res = run_bass_kernel_spmd(nc, in_maps, core_ids=list(range(N_CORES)))
    out = np.empty((B, N, D), dtype=np.float32)
    for c in range(N_CORES):
        b, chunk = divmod(c, per_batch)
        out[b, chunk * NQ:(chunk + 1) * NQ, :] = res.results[c]["yT"].T
    return out

